# revision 1
# baseline (speedup 1.0000x reference)
"""DiffAttn3d Trainium2 kernel.

8-core sharding: core c -> (batch b = c//4, query slice qs = (c%4)*512).
Each core computes its 512-query slice of the full differential-attention
block (all 16 n-heads) and the final output projection for that slice.

Flash-style: scores are computed transposed (keys on partitions) so the
AV matmul needs no transposes of the big E matrix. The boolean mask is
folded into the score PSUM via a 60*I @ maskT matmul preload and a -60
exp bias (masked entries become exp(s-60) ~ 1e-26 ~ 0). All matmul
operands are bf16 (fp32 moving operands stream at half rate on PE);
accumulation stays fp32 in PSUM. RMSNorm's sqrt is computed as
exp(-0.5*ln(ms)) so the ACT engine never switches activation tables
(ln+exp live in one table set), and the 1/rms scale is applied after
the output projection (it commutes with the per-head matmul).
"""

import math
import numpy as np

B, L, IN_DIM, OUT_DIM = 2, 2048, 128, 128
H, DH = 8, 32
ED = H * DH * 2          # 512
NH = 2 * H               # 16 n-heads
DEPTH = 1
LAMBDA_INIT = 0.8 - 0.6 * math.exp(-0.3 * (DEPTH + 1))
EPS = 1e-8

QSL = 512                # queries per core
NKC = L // 128           # 16 key chunks
NSC = L // 512           # 4 seq chunks of 512
NQS = QSL // 128         # 4 query subtiles
NCH = 6                  # ED chunks for qT/kT: 3 heads per 128 partitions
MASK_BIG = 60.0

_CACHE = {}


def _build_program():
    import concourse.bass as bass
    import concourse.tile as tile
    from concourse import bacc, mybir
    from concourse.masks import make_identity

    f32 = mybir.dt.float32
    bf16 = mybir.dt.bfloat16
    u8 = mybir.dt.uint8
    u32 = mybir.dt.uint32
    AF = mybir.ActivationFunctionType
    ALU = mybir.AluOpType

    nc = bacc.Bacc("TRN2", target_bir_lowering=False, debug=False,
                   num_devices=8)

    xs_d = nc.declare_dram_parameter("xs", [L, IN_DIM], f32, isOutput=False)
    xq_d = nc.declare_dram_parameter("xq", [QSL, IN_DIM], f32, isOutput=False)
    mT_d = nc.declare_dram_parameter("maskT", [L, QSL], u8, isOutput=False)
    # w = [Wq padded to 6 chunks | Wk padded to 6 chunks | Wv] (see host prep)
    w_d = nc.declare_dram_parameter("w", [IN_DIM, NCH * 256 + ED], f32,
                                    isOutput=False)
    wo_d = nc.declare_dram_parameter("wo", [ED, OUT_DIM], f32, isOutput=False)
    nlam_d = nc.declare_dram_parameter("nlam", [128, 1], f32, isOutput=False)
    out_d = nc.declare_dram_parameter("out", [QSL, OUT_DIM], f32, isOutput=True)

    with tile.TileContext(nc) as tc:
        with (
            tc.tile_pool(name="const", bufs=1) as const,
            tc.tile_pool(name="xin", bufs=3) as xin_p,
            tc.tile_pool(name="psA", bufs=2, space=bass.MemorySpace.PSUM) as psA,
            tc.tile_pool(name="avp", bufs=1, space=bass.MemorySpace.PSUM) as avp,
            tc.tile_pool(name="trp", bufs=2, space=bass.MemorySpace.PSUM) as trp,
            tc.tile_pool(name="epool", bufs=3) as epool,
            tc.tile_pool(name="natp", bufs=2) as natp,
            tc.tile_pool(name="tinyp", bufs=8) as tinyp,
            tc.tile_pool(name="tmpp", bufs=4) as tmpp,
        ):
            # ---- constants / weights ----
            w_sb = const.tile([128, NCH * 256 + ED], f32)
            nc.sync.dma_start(w_sb[:], w_d[:])
            w_bf = const.tile([128, NCH * 256 + ED], bf16)
            nc.vector.tensor_copy(w_bf[:], w_sb[:])
            wo_sb = const.tile([64, H, 128], f32)
            nc.sync.dma_start(wo_sb[:], wo_d.rearrange("(t p) o -> p t o", p=64))
            wo_bf = const.tile([64, H, 128], bf16)
            nc.vector.tensor_copy(wo_bf[:], wo_sb[:])
            nlam_sb = const.tile([128, 1], f32)
            nc.sync.dma_start(nlam_sb[:], nlam_d[:])

            eye1 = const.tile([128, 128], f32)
            make_identity(nc, eye1[:])
            eyebf = const.tile([128, 128], bf16)
            nc.vector.tensor_copy(eyebf[:], eye1[:])
            eye60 = const.tile([128, 128], bf16)
            nc.vector.tensor_scalar_mul(eye60[:], eye1[:], MASK_BIG)
            negbig = const.tile([128, 1], f32)
            nc.vector.memset(negbig[:], -MASK_BIG)
            magic = const.tile([128, NQS], u32)
            nc.vector.memset(magic[:], 0x5F3759DF)

            # ---- xsT: transpose x [L,128] -> [128, L] (bf16 out) ----
            xsT = const.tile([128, L], bf16)
            for s4 in range(L // 512):
                ps = psA.tile([128, 2, 512], f32, tag="sps", name="ps")
                for t in range(4):
                    st = s4 * 4 + t
                    xin = xin_p.tile([128, 128], f32, tag="xin")
                    nc.sync.dma_start(xin[:], xs_d[st * 128:(st + 1) * 128, :])
                    nc.tensor.transpose(ps[:, 0, t * 128:(t + 1) * 128],
                                        xin[:], eye1[:])
                nc.scalar.copy(xsT[:, s4 * 512:(s4 + 1) * 512], ps[:, 0, :])
            xqT = const.tile([128, QSL], bf16)
            ps = psA.tile([128, 2, 512], f32, tag="sps", name="ps")
            for st in range(QSL // 128):
                xin = xin_p.tile([128, 128], f32, tag="xin")
                nc.sync.dma_start(xin[:], xq_d[st * 128:(st + 1) * 128, :])
                nc.tensor.transpose(ps[:, 0, st * 128:(st + 1) * 128],
                                    xin[:], eye1[:])
            nc.scalar.copy(xqT[:], ps[:, 0, :])

            # ---- projections (bf16 in/out, fp32 psum) ----
            # qT[c] = (Wq_pad[:, c*128:+128]).T @ xqT  -> [128, QSL]
            # chunk c holds heads 3c..3c+2 at partition offsets 0/32/64
            qT = const.tile([128, NCH, QSL], bf16)
            for c in range(NCH):
                ps = psA.tile([128, QSL], f32, tag="sps", name="ps")
                nc.tensor.matmul(ps[:], w_bf[:, c * 128:(c + 1) * 128], xqT[:],
                                 start=True, stop=True)
                if c % 2:
                    nc.scalar.copy(qT[:, c, :], ps[:])
                else:
                    nc.vector.tensor_copy(qT[:, c, :], ps[:])
            # kT[c] = (Wk_pad[:, c*128:+128]).T @ xsT -> [128, L]
            kT = const.tile([128, NCH, L], bf16)
            for c in range(NCH):
                for s in range(NSC):
                    ps = psA.tile([128, 512], f32, tag="sps", name="ps")
                    nc.tensor.matmul(
                        ps[:],
                        w_bf[:, NCH * 128 + c * 128:NCH * 128 + (c + 1) * 128],
                        xsT[:, s * 512:(s + 1) * 512], start=True, stop=True)
                    if s % 2:
                        nc.scalar.copy(kT[:, c, s * 512:(s + 1) * 512], ps[:])
                    else:
                        nc.vector.tensor_copy(
                            kT[:, c, s * 512:(s + 1) * 512], ps[:])
            # v+ones: vp[st, h, 0:64] = (xs @ Wv)[st, h*64:(h+1)*64]; vp[st,h,64]=1
            vp = const.tile([128, NKC, H, 65], bf16)
            for st in range(NKC):
                ps = psA.tile([128, 512], f32, tag="sps", name="ps")
                nc.tensor.matmul(ps[:], xsT[:, st * 128:(st + 1) * 128],
                                 w_bf[:, NCH * 256:NCH * 256 + ED],
                                 start=True, stop=True)
                if st % 2:
                    nc.scalar.copy(vp[:, st, :, 0:64],
                                   ps[:].rearrange("p (h d) -> p h d", h=H))
                else:
                    nc.vector.tensor_copy(
                        vp[:, st, :, 0:64],
                        ps[:].rearrange("p (h d) -> p h d", h=H))
            nc.vector.memset(vp[:, :, :, 64:65], 1.0)

            # ---- mask -> bf16 ----
            mkf = const.tile([128, NKC, QSL], bf16)
            for kc in range(NKC):
                mu = xin_p.tile([128, QSL], u8, tag="mu8")
                nc.sync.dma_start(mu[:], mT_d[kc * 128:(kc + 1) * 128, :])
                if kc % 2:
                    nc.scalar.copy(mkf[:, kc, :], mu[:])
                else:
                    nc.vector.tensor_copy(mkf[:, kc, :], mu[:])

            # ---- attention: 8 passes, one H-pair (2 n-heads) each ----
            out_acc = const.tile([128, NQS, 128], f32)
            for g in range(H):           # H-pair / pass index
                av = [avp.tile([65, QSL], f32, tag=f"av{j}", name=f"av{j}")
                      for j in range(2)]
                for kc in range(NKC):
                    sps = psA.tile([128, 2, QSL], f32, tag="sps", name="sps")
                    for j in range(2):
                        n = 2 * g + j
                        c, r = n // 3, (n % 3) * 32
                        nc.tensor.matmul(sps[:, j, :], eye60[:], mkf[:, kc, :],
                                         start=True, stop=False)
                        nc.tensor.matmul(
                            sps[:, j, :],
                            kT[r:r + 32, c, kc * 128:(kc + 1) * 128],
                            qT[r:r + 32, c, :], start=False, stop=True)
                    e = epool.tile([128, 2, QSL], bf16, tag="e")
                    nc.scalar.activation(e[:], sps[:], AF.Exp, bias=negbig[:])
                    for j in range(2):
                        nc.tensor.matmul(av[j][:], vp[:, kc, g, :], e[:, j, :],
                                         start=(kc == 0), stop=(kc == NKC - 1))

                # epilogue for the pair: back to natural [q, 65] layout
                nat = []
                for j in range(2):
                    a_sb = tmpp.tile([65, QSL], f32, tag="a_sb")
                    nc.scalar.copy(a_sb[:], av[j][:])
                    tp = trp.tile([128, NQS, 65], f32, tag="trp")
                    for q in range(NQS):
                        nc.tensor.transpose(
                            tp[:, q, :], a_sb[:, q * 128:(q + 1) * 128],
                            eye1[0:65, 0:65])
                    nt = natp.tile([128, NQS, 65], f32, tag=f"nat{j}",
                                   name=f"nat{j}")
                    nc.scalar.copy(nt[:], tp[:])
                    nat.append(nt)

                # batched reciprocals of the two denominator columns
                r0v = tinyp.tile([128, NQS, 1], f32, tag="r0v")
                nc.vector.reciprocal(r0v[:], nat[0][:, :, 64:65])
                r1v = tinyp.tile([128, NQS, 1], f32, tag="r1v")
                nc.vector.reciprocal(r1v[:], nat[1][:, :, 64:65])
                r1p = tinyp.tile([128, NQS, 1], f32, tag="r1p")
                nc.vector.tensor_scalar(r1p[:], r1v[:], nlam_sb[:], None,
                                        ALU.mult)

                op = trp.tile([128, NQS, 128], f32, tag="trp", name="op")
                ss4 = tinyp.tile([128, NQS], f32, tag="ss4")
                ats = []
                for q in range(NQS):
                    t0 = tmpp.tile([128, 64], f32, tag="t0")
                    nc.vector.tensor_scalar(t0[:], nat[0][:, q, 0:64],
                                            r0v[:, q, :], None, ALU.mult)
                    at = tmpp.tile([128, 64], bf16, tag=f"at{q}",
                                   name=f"at{q}")
                    nc.vector.scalar_tensor_tensor(
                        at[:], nat[1][:, q, 0:64], r1p[:, q, :], t0[:],
                        ALU.mult, ALU.add)
                    ats.append(at)
                    # mean-square accumulated per qsub into ss4
                    sqj = tmpp.tile([128, 64], bf16, tag="sqj")
                    nc.vector.scalar_tensor_tensor(
                        sqj[:], at[:], 1.0, at[:], ALU.bypass, ALU.mult,
                        accum_out=ss4[:, q:q + 1])
                # rr4 = 1/sqrt(ss4/64): fast inverse sqrt, all on DVE
                msx = tinyp.tile([128, NQS], f32, tag="msx")
                nc.vector.tensor_scalar(msx[:], ss4[:], 1.0 / 64, None,
                                        ALU.mult)
                sh = tinyp.tile([128, NQS], u32, tag="sh")
                nc.vector.tensor_scalar(sh[:], msx[:].bitcast(u32), 1, None,
                                        ALU.logical_shift_right)
                rr4 = tinyp.tile([128, NQS], f32, tag="rr4")
                nc.vector.tensor_tensor(rr4[:].bitcast(u32), magic[:], sh[:],
                                        ALU.subtract)
                nwu = tinyp.tile([128, NQS], f32, tag="nwu")
                nww = tinyp.tile([128, NQS], f32, tag="nww")
                for _ in range(2):
                    nc.vector.tensor_tensor(nwu[:], rr4[:], rr4[:], ALU.mult)
                    nc.vector.scalar_tensor_tensor(
                        nwu[:], nwu[:], 0.5, msx[:], ALU.mult, ALU.mult)
                    nc.vector.tensor_scalar(nww[:], nwu[:], -1.0, 1.5,
                                            ALU.mult, ALU.add)
                    nc.vector.tensor_tensor(rr4[:], rr4[:], nww[:], ALU.mult)
                for q in range(NQS):
                    # transpose [128,64] -> [64,128] and project
                    tq = trp.tile([64, 128], bf16, tag="trp", name="tq")
                    nc.tensor.transpose(tq[:], ats[q][:], eyebf[:])
                    atT = tmpp.tile([64, 128], bf16, tag="atT")
                    nc.vector.tensor_copy(atT[:], tq[:])
                    nc.tensor.matmul(op[:, q, :], atT[:], wo_bf[:, g, :],
                                     start=True, stop=True)
                    # out_acc += rr * op   (1/rms commutes with the matmul)
                    if g == 0:
                        nc.vector.tensor_scalar(out_acc[:, q, :], op[:, q, :],
                                                rr4[:, q:q + 1], None,
                                                ALU.mult)
                    else:
                        nc.vector.scalar_tensor_tensor(
                            out_acc[:, q, :], op[:, q, :], rr4[:, q:q + 1],
                            out_acc[:, q, :], ALU.mult, ALU.add)

            nc.sync.dma_start(out_d.rearrange("(s p) o -> p s o", p=128),
                              out_acc[:])

    nc.compile()
    return nc


def kernel(**inputs):
    from concourse.bass_utils import run_bass_kernel_spmd

    x = np.asarray(inputs["x"], np.float32)
    mask = np.asarray(inputs["mask_2d"]).astype(np.uint8)
    Wq = np.asarray(inputs["Wq"], np.float32)
    Wkv = np.asarray(inputs["Wkv"], np.float32)
    Wout = np.asarray(inputs["Wout"], np.float32)
    lq1 = np.asarray(inputs["lambda_q1"], np.float32)
    lk1 = np.asarray(inputs["lambda_k1"], np.float32)
    lq2 = np.asarray(inputs["lambda_q2"], np.float32)
    lk2 = np.asarray(inputs["lambda_k2"], np.float32)
    gamma = np.asarray(inputs["gamma"], np.float32)

    lam = float(np.exp(np.sum(lq1 * lk1)) - np.exp(np.sum(lq2 * lk2))
                + LAMBDA_INIT)
    Wq_s = (Wq * DH ** -0.5).astype(np.float32)
    Wk = Wkv[:, :ED]
    Wv = Wkv[:, ED:]

    def pad_heads(Wm):
        # chunk c (128 cols) holds heads 3c..3c+2 at col offsets 0/32/64
        out = np.zeros((IN_DIM, NCH * 128), np.float32)
        for n in range(NH):
            c, r = divmod(n, 3)
            out[:, c * 128 + r * 32:c * 128 + r * 32 + 32] = \
                Wm[:, n * DH:(n + 1) * DH]
        return out

    W = np.ascontiguousarray(
        np.concatenate([pad_heads(Wq_s), pad_heads(Wk), Wv], axis=1))
    gs = (gamma * (1.0 - LAMBDA_INIT)).astype(np.float32)
    Wog = np.ascontiguousarray(Wout * np.tile(gs, H)[:, None])
    nlam = np.full((128, 1), -lam, np.float32)

    maskT = [np.ascontiguousarray(mask[b].T) for b in range(B)]

    if "nc" not in _CACHE:
        _CACHE["nc"] = _build_program()
    nc = _CACHE["nc"]

    in_maps = []
    for c in range(8):
        b, qc = divmod(c, 4)
        in_maps.append({
            "xs": np.ascontiguousarray(x[b, 0]),
            "xq": np.ascontiguousarray(x[b, 0, qc * QSL:(qc + 1) * QSL, :]),
            "maskT": np.ascontiguousarray(maskT[b][:, qc * QSL:(qc + 1) * QSL]),
            "w": W,
            "wo": Wog,
            "nlam": nlam,
        })

    r = run_bass_kernel_spmd(nc, in_maps, list(range(8)))
    _CACHE["last_results"] = r
    res = r.results

    out = np.empty((B, 1, L, OUT_DIM), np.float32)
    for c in range(8):
        b, qc = divmod(c, 4)
        out[b, 0, qc * QSL:(qc + 1) * QSL, :] = res[c]["out"]
    return out



# revision 6
# speedup vs baseline: 1.1652x; 1.1652x over previous
"""DiffAttn3d Trainium2 kernel (v3).

8-core sharding: core c -> (batch b = c//4, query slice qs = (c%4)*512).
Each core computes its 512-query slice of the full differential-attention
block (all 16 n-heads) and the final output projection for that slice.

Structure:
- Host pre-transposes x (xsT/xqT bf16) and pre-packs all weights in bf16;
  the kernel has no input-transpose or weight-cast phase.
- Heads packed 4-per-128-partitions (offsets 0/32/64/96); the two QK^T
  matmuls of a head pair use explicit PE row tiling (tile_position) and
  run concurrently on their two 32-row groups.
- Scores live in a double-buffered [128, 2, 512] PSUM tile so the next
  key chunk's QK overlaps the current exp; the ACT engine streams
  [128, 1024] exp batches back to back (ACT is the critical engine).
- The boolean mask is applied multiplicatively on the Vector engine
  (bf16 tensor_tensor with a broadcast AP) after the exp.
- AV accumulates [80, 512] per n-head in PSUM (ones columns 64:80 give
  the softmax denominator); the flash-style transposed layout means the
  AV matmuls need no transposes of e.
- Epilogue per pair: PSUM->SBUF bf16 copy, DMA-engine xbar transposes
  back to natural [q, 80] layout (PE/ACT stay off the epilogue), batched
  DVE combine + fast inverse sqrt, PE transpose of the 64-dim attn rows,
  and output projection accumulated across all 8 pairs in one persistent
  PSUM bank.
PSUM budget: 4 (scores x2) + 2 (av) + 1 (out proj) + 1 (PE transpose) = 8.
"""

import math
import numpy as np

B, L, IN_DIM, OUT_DIM = 2, 2048, 128, 128
H, DH = 8, 32
ED = H * DH * 2          # 512
NH = 2 * H               # 16 n-heads
DEPTH = 1
LAMBDA_INIT = 0.8 - 0.6 * math.exp(-0.3 * (DEPTH + 1))
EPS = 1e-8

QSL = 512                # queries per core
NKC = L // 128           # 16 key chunks
NQS = QSL // 128         # 4 query subtiles
NCH = 4                  # head chunks: 4 heads per 128 partitions
AVP = 80                 # av rows: 64 v-dims + 16 ones (xbar tile = 16 rows)

_CACHE = {}


def _build_program():
    import concourse.bass as bass
    import concourse.tile as tile
    from concourse import bacc, mybir
    from concourse.masks import make_identity

    f32 = mybir.dt.float32
    bf16 = mybir.dt.bfloat16
    u32 = mybir.dt.uint32
    AF = mybir.ActivationFunctionType
    ALU = mybir.AluOpType

    nc = bacc.Bacc("TRN2", target_bir_lowering=False, debug=False,
                   num_devices=8)

    xsT_d = nc.declare_dram_parameter("xsT", [IN_DIM, L], bf16, isOutput=False)
    xqT_d = nc.declare_dram_parameter("xqT", [IN_DIM, QSL], bf16,
                                      isOutput=False)
    mT_d = nc.declare_dram_parameter("maskT", [L, QSL], bf16, isOutput=False)
    # w = [Wq4 | Wk4 | Wv]: Wq/Wk packed 4 heads per 128 cols (offsets
    # 0/32/64/96), Wq pre-scaled by DH^-0.5.
    w_d = nc.declare_dram_parameter("w", [IN_DIM, 3 * ED], bf16,
                                    isOutput=False)
    wo_d = nc.declare_dram_parameter("wo", [64, H, OUT_DIM], bf16,
                                     isOutput=False)
    nlam_d = nc.declare_dram_parameter("nlam", [128, 1], f32, isOutput=False)
    out_d = nc.declare_dram_parameter("out", [QSL, OUT_DIM], f32, isOutput=True)

    with tile.TileContext(nc) as tc:
        with (
            tc.tile_pool(name="const", bufs=1) as const,
            tc.tile_pool(name="psA", bufs=2, space=bass.MemorySpace.PSUM) as psA,
            tc.tile_pool(name="avp", bufs=1, space=bass.MemorySpace.PSUM) as avp,
            tc.tile_pool(name="outp", bufs=1, space=bass.MemorySpace.PSUM) as outp,
            tc.tile_pool(name="trp", bufs=1, space=bass.MemorySpace.PSUM) as trp,
            tc.tile_pool(name="epool", bufs=3) as epool,
            tc.tile_pool(name="natp", bufs=2) as natp,
            tc.tile_pool(name="tinyp", bufs=4) as tinyp,
            tc.tile_pool(name="tmpp", bufs=4) as tmpp,
        ):
            # ---- constants / weights (all DMA'd pre-packed in bf16) ----
            w_sb = const.tile([128, 3 * ED], bf16)
            nc.sync.dma_start(w_sb[:], w_d[:])
            wo_sb = const.tile([64, H, OUT_DIM], bf16)
            nc.sync.dma_start(wo_sb[:], wo_d[:])
            nlam_sb = const.tile([128, 1], f32)
            nc.sync.dma_start(nlam_sb[:], nlam_d[:])
            xsT = const.tile([128, L], bf16)
            nc.sync.dma_start(xsT[:], xsT_d[:])
            xqT = const.tile([128, QSL], bf16)
            nc.sync.dma_start(xqT[:], xqT_d[:])
            mkf = const.tile([128, NKC, QSL], bf16)
            nc.sync.dma_start(mkf[:], mT_d.rearrange("(kc p) q -> p kc q",
                                                     p=128))

            eye1 = const.tile([128, 128], f32)
            make_identity(nc, eye1[:])
            eyebf = const.tile([128, 128], bf16)
            nc.vector.tensor_copy(eyebf[:], eye1[:])
            magic = const.tile([128, NQS], u32)
            nc.vector.memset(magic[:], 0x5F3759DF)

            # ---- projections (bf16 in/out, fp32 psum) ----
            qT = const.tile([128, NCH, QSL], bf16)
            kT = const.tile([128, NCH, L], bf16)
            vp = const.tile([128, NKC, H, AVP], bf16)
            for c2 in range(2):
                ps = psA.tile([128, 2, QSL], f32, tag="sps", name="ps")
                for s in range(2):
                    c = c2 * 2 + s
                    nc.tensor.matmul(ps[:, s, :],
                                     w_sb[:, c * 128:(c + 1) * 128],
                                     xqT[:], start=True, stop=True)
                nc.scalar.copy(qT[:, c2 * 2:(c2 + 1) * 2, :], ps[:])
            for c in range(NCH):
                for h2 in range(2):
                    ps = psA.tile([128, 2, QSL], f32, tag="sps", name="ps")
                    for s in range(2):
                        nc.tensor.matmul(
                            ps[:, s, :],
                            w_sb[:, ED + c * 128:ED + (c + 1) * 128],
                            xsT[:, (h2 * 2 + s) * 512:(h2 * 2 + s + 1) * 512],
                            start=True, stop=True)
                    if h2:
                        nc.scalar.copy(
                            kT[:, c, h2 * 1024:(h2 + 1) * 1024],
                            ps[:].rearrange("p s q -> p (s q)"))
                    else:
                        nc.vector.tensor_copy(
                            kT[:, c, h2 * 1024:(h2 + 1) * 1024],
                            ps[:].rearrange("p s q -> p (s q)"))
            for st2 in range(NKC // 2):
                ps = psA.tile([128, 2, QSL], f32, tag="sps", name="ps")
                for s in range(2):
                    st = st2 * 2 + s
                    nc.tensor.matmul(ps[:, s, :],
                                     xsT[:, st * 128:(st + 1) * 128],
                                     w_sb[:, 2 * ED:3 * ED],
                                     start=True, stop=True)
                for s in range(2):
                    st = st2 * 2 + s
                    if s:
                        nc.scalar.copy(
                            vp[:, st, :, 0:64],
                            ps[:, s, :].rearrange("p (h d) -> p h d", h=H))
                    else:
                        nc.vector.tensor_copy(
                            vp[:, st, :, 0:64],
                            ps[:, s, :].rearrange("p (h d) -> p h d", h=H))
            nc.vector.memset(vp[:, :, :, 64:AVP], 1.0)

            out_ps = outp.tile([128, NQS, 128], f32, tag="op", name="out_ps")

            # ---- attention: 8 pairs (2 n-heads each), streaming ----
            for c in range(NCH):
                for p in range(2):
                    g = 2 * c + p
                    av = [avp.tile([AVP, QSL], f32, tag=f"av{j}",
                                   name=f"av{j}") for j in range(2)]
                    for kc in range(NKC):
                        sps = psA.tile([128, 2, QSL], f32, tag="sps",
                                       name="sps")
                        for j in range(2):
                            r = 64 * p + 32 * j
                            nc.tensor.matmul(
                                sps[:, j, :],
                                kT[r:r + 32, c, kc * 128:(kc + 1) * 128],
                                qT[r:r + 32, c, :],
                                start=True, stop=True,
                                tile_position=(r, 0))
                        e = epool.tile([128, 2, QSL], bf16, tag="e")
                        nc.scalar.activation(e[:], sps[:], AF.Exp)
                        em = epool.tile([128, 2, QSL], bf16, tag="em")
                        mb = mkf[:, kc, :].unsqueeze(1).broadcast_to(
                            [128, 2, QSL])
                        nc.vector.tensor_tensor(em[:], e[:], mb, ALU.mult)
                        for j in range(2):
                            nc.tensor.matmul(av[j][:], vp[:, kc, g, :],
                                             em[:, j, :],
                                             start=(kc == 0),
                                             stop=(kc == NKC - 1))

                    # ---- epilogue: back to natural [q, d] layout ----
                    nat = []
                    for j in range(2):
                        a_sb = tmpp.tile([AVP, QSL], bf16, tag="a_sb")
                        nc.vector.tensor_copy(a_sb[:], av[j][:])
                        nt = natp.tile([128, NQS, AVP], bf16, tag=f"nat{j}",
                                       name=f"nat{j}")
                        for q in range(NQS):
                            nc.sync.dma_start_transpose(
                                nt[:, q, :], a_sb[:, q * 128:(q + 1) * 128])
                        nat.append(nt)

                    r0v = tinyp.tile([128, NQS, 1], f32, tag="r0v")
                    nc.vector.reciprocal(r0v[:], nat[0][:, :, 64:65])
                    r1v = tinyp.tile([128, NQS, 1], f32, tag="r1v")
                    nc.vector.reciprocal(r1v[:], nat[1][:, :, 64:65])
                    r1p = tinyp.tile([128, NQS, 1], f32, tag="r1p")
                    nc.vector.tensor_scalar(r1p[:], r1v[:], nlam_sb[:], None,
                                            ALU.mult)

                    t0 = tmpp.tile([128, NQS, 64], f32, tag="t0")
                    nc.vector.tensor_tensor(
                        t0[:], nat[0][:, :, 0:64],
                        r0v[:].broadcast_to([128, NQS, 64]), ALU.mult)
                    t1 = tmpp.tile([128, NQS, 64], f32, tag="t1")
                    nc.vector.tensor_tensor(
                        t1[:], nat[1][:, :, 0:64],
                        r1p[:].broadcast_to([128, NQS, 64]), ALU.mult)
                    at4 = tmpp.tile([128, NQS, 64], bf16, tag="at4")
                    nc.vector.tensor_tensor(at4[:], t0[:], t1[:], ALU.add)
                    sq4 = tmpp.tile([128, NQS, 64], f32, tag="sq4")
                    nc.vector.tensor_tensor(sq4[:], at4[:], at4[:], ALU.mult)
                    ss4 = tinyp.tile([128, NQS], f32, tag="ss4")
                    nc.vector.tensor_reduce(ss4[:], sq4[:],
                                            mybir.AxisListType.X, ALU.add)

                    # rr4 = 1/sqrt(ss4/64): fast inverse sqrt on DVE
                    msx = tinyp.tile([128, NQS], f32, tag="msx")
                    nc.vector.tensor_scalar(msx[:], ss4[:], 1.0 / 64, None,
                                            ALU.mult)
                    sh = tinyp.tile([128, NQS], u32, tag="sh")
                    nc.vector.tensor_scalar(sh[:], msx[:].bitcast(u32), 1,
                                            None, ALU.logical_shift_right)
                    rr4 = tinyp.tile([128, NQS], f32, tag="rr4")
                    nc.vector.tensor_tensor(rr4[:].bitcast(u32), magic[:],
                                            sh[:], ALU.subtract)
                    nwu = tinyp.tile([128, NQS], f32, tag="nwu")
                    nww = tinyp.tile([128, NQS], f32, tag="nww")
                    for _ in range(2):
                        nc.vector.tensor_tensor(nwu[:], rr4[:], rr4[:],
                                                ALU.mult)
                        nc.vector.scalar_tensor_tensor(
                            nwu[:], nwu[:], 0.5, msx[:], ALU.mult, ALU.mult)
                        nc.vector.tensor_scalar(nww[:], nwu[:], -1.0, 1.5,
                                                ALU.mult, ALU.add)
                        nc.vector.tensor_tensor(rr4[:], rr4[:], nww[:],
                                                ALU.mult)

                    at_s = tmpp.tile([128, NQS, 64], bf16, tag="at_s")
                    nc.vector.tensor_tensor(
                        at_s[:], at4[:],
                        rr4[:].unsqueeze(2).broadcast_to([128, NQS, 64]),
                        ALU.mult)
                    for q in range(NQS):
                        tq = trp.tile([64, 128], bf16, tag="tq", name="tq")
                        nc.tensor.transpose(tq[:], at_s[:, q, :], eyebf[:])
                        atT = tmpp.tile([64, 128], bf16, tag="atT")
                        nc.vector.tensor_copy(atT[:], tq[:])
                        # one accumulation group for the whole bank: start
                        # clears has_written bank-wide, so only the very
                        # first matmul may carry it
                        nc.tensor.matmul(out_ps[:, q, :], atT[:],
                                         wo_sb[:, g, :],
                                         start=(g == 0 and q == 0),
                                         stop=(g == H - 1 and q == NQS - 1))

            out_sb = const.tile([128, NQS, 128], f32)
            nc.vector.tensor_copy(out_sb[:], out_ps[:])
            nc.sync.dma_start(out_d.rearrange("(s p) o -> p s o", p=128),
                              out_sb[:])

    nc.compile()
    return nc


def kernel(**inputs):
    import ml_dtypes
    from concourse.bass_utils import run_bass_kernel_spmd

    bfdt = ml_dtypes.bfloat16

    x = np.asarray(inputs["x"], np.float32)
    mask = np.asarray(inputs["mask_2d"])
    Wq = np.asarray(inputs["Wq"], np.float32)
    Wkv = np.asarray(inputs["Wkv"], np.float32)
    Wout = np.asarray(inputs["Wout"], np.float32)
    lq1 = np.asarray(inputs["lambda_q1"], np.float32)
    lk1 = np.asarray(inputs["lambda_k1"], np.float32)
    lq2 = np.asarray(inputs["lambda_q2"], np.float32)
    lk2 = np.asarray(inputs["lambda_k2"], np.float32)
    gamma = np.asarray(inputs["gamma"], np.float32)

    lam = float(np.exp(np.sum(lq1 * lk1)) - np.exp(np.sum(lq2 * lk2))
                + LAMBDA_INIT)
    Wq_s = (Wq * DH ** -0.5).astype(np.float32)
    Wk = Wkv[:, :ED]
    Wv = Wkv[:, ED:]

    def pack_heads4(Wm):
        # chunk c (128 cols) holds heads 4c..4c+3 at col offsets 0/32/64/96
        out = np.empty((IN_DIM, NCH * 128), np.float32)
        for n in range(NH):
            c, r = divmod(n, 4)
            out[:, c * 128 + r * 32:c * 128 + r * 32 + 32] = \
                Wm[:, n * DH:(n + 1) * DH]
        return out

    W = np.concatenate([pack_heads4(Wq_s), pack_heads4(Wk), Wv],
                       axis=1).astype(bfdt)
    gs = (gamma * (1.0 - LAMBDA_INIT)).astype(np.float32)
    Wog = (Wout * np.tile(gs, H)[:, None])
    wo = np.ascontiguousarray(
        Wog.reshape(H, 64, OUT_DIM).transpose(1, 0, 2)).astype(bfdt)
    nlam = np.full((128, 1), -lam, np.float32)

    xsT = [np.ascontiguousarray(x[b, 0].T).astype(bfdt) for b in range(B)]
    maskT = [np.ascontiguousarray(mask[b].T.astype(np.float32)).astype(bfdt)
             for b in range(B)]

    if "nc" not in _CACHE:
        _CACHE["nc"] = _build_program()
    nc = _CACHE["nc"]

    in_maps = []
    for core in range(8):
        b, qc = divmod(core, 4)
        in_maps.append({
            "xsT": xsT[b],
            "xqT": np.ascontiguousarray(
                xsT[b][:, qc * QSL:(qc + 1) * QSL]),
            "maskT": np.ascontiguousarray(
                maskT[b][:, qc * QSL:(qc + 1) * QSL]),
            "w": W,
            "wo": wo,
            "nlam": nlam,
        })

    r = run_bass_kernel_spmd(nc, in_maps, list(range(8)))
    _CACHE["last_results"] = r
    res = r.results

    out = np.empty((B, 1, L, OUT_DIM), np.float32)
    for core in range(8):
        b, qc = divmod(core, 4)
        out[b, 0, qc * QSL:(qc + 1) * QSL, :] = res[core]["out"]
    return out


# revision 7
# speedup vs baseline: 1.2027x; 1.0322x over previous
"""DiffAttn3d Trainium2 kernel (v4).

8-core sharding: core c -> (batch b = c//4, query slice qs = (c%4)*512).
Each core computes its 512-query slice of the full differential-attention
block (all 16 n-heads) and the final output projection for that slice.

Structure:
- Host pre-transposes x (xsT/xqT bf16) and pre-packs all weights in bf16.
- Heads packed 4-per-128-partitions (offsets 0/32/64/96); the two QK^T
  matmuls of a head pair use explicit PE row tiling and run concurrently
  on their two 32-row groups.
- Software-pipelined at the head-pair level: phase i streams QK -> exp
  ([128,1024] ACT batches from a double-buffered 2-bank score tile) ->
  multiplicative bf16 mask (DVE, broadcast AP) into an SBUF e-buffer for
  pair i, while the SAME phase's PE queue carries the AV matmuls of pair
  i-1 reading the previous e-buffer. Engine queues are strict FIFO, so
  this interleaved emission is what keeps the PE busy during exp and the
  ACT stream dense (and the PE HAM-warm).
- AV accumulates [80, 512] per n-head in PSUM (ones columns 64:80 give
  the softmax denominator).
- Epilogue per pair: PSUM->SBUF bf16 copy, DMA-engine xbar transposes
  to natural [q, d] layout, batched DVE combine + fast inverse sqrt,
  DMA transpose of the scaled attn rows (padded to 128 cols; junk cols
  are never read), and the output projection accumulated across all 8
  pairs in one persistent PSUM bank (single accumulation group - start
  clears has_written bank-wide).
PSUM budget: 4 (scores x2) + 2 (av) + 1 (out proj) = 7 of 8 banks.
"""

import math
import numpy as np

B, L, IN_DIM, OUT_DIM = 2, 2048, 128, 128
H, DH = 8, 32
ED = H * DH * 2          # 512
NH = 2 * H               # 16 n-heads
DEPTH = 1
LAMBDA_INIT = 0.8 - 0.6 * math.exp(-0.3 * (DEPTH + 1))
EPS = 1e-8

QSL = 512                # queries per core
NKC = L // 128           # 16 key chunks
NQS = QSL // 128         # 4 query subtiles
NCH = 4                  # head chunks: 4 heads per 128 partitions
AVP = 80                 # av rows: 64 v-dims + 16 ones (xbar tile = 16 rows)

_CACHE = {}


def _build_program():
    import concourse.bass as bass
    import concourse.tile as tile
    from concourse import bacc, mybir

    f32 = mybir.dt.float32
    bf16 = mybir.dt.bfloat16
    u32 = mybir.dt.uint32
    AF = mybir.ActivationFunctionType
    ALU = mybir.AluOpType

    nc = bacc.Bacc("TRN2", target_bir_lowering=False, debug=False,
                   num_devices=8)

    xsT_d = nc.declare_dram_parameter("xsT", [IN_DIM, L], bf16, isOutput=False)
    xqT_d = nc.declare_dram_parameter("xqT", [IN_DIM, QSL], bf16,
                                      isOutput=False)
    mT_d = nc.declare_dram_parameter("maskT", [L, QSL], bf16, isOutput=False)
    # w = [Wq4 | Wk4 | Wv]: Wq/Wk packed 4 heads per 128 cols (offsets
    # 0/32/64/96), Wq pre-scaled by DH^-0.5.
    w_d = nc.declare_dram_parameter("w", [IN_DIM, 3 * ED], bf16,
                                    isOutput=False)
    wo_d = nc.declare_dram_parameter("wo", [64, H, OUT_DIM], bf16,
                                     isOutput=False)
    nlam_d = nc.declare_dram_parameter("nlam", [128, 1], f32, isOutput=False)
    out_d = nc.declare_dram_parameter("out", [QSL, OUT_DIM], f32, isOutput=True)

    with tile.TileContext(nc) as tc:
        with (
            tc.tile_pool(name="const", bufs=1) as const,
            tc.tile_pool(name="psA", bufs=2, space=bass.MemorySpace.PSUM) as psA,
            tc.tile_pool(name="avp", bufs=1, space=bass.MemorySpace.PSUM) as avp,
            tc.tile_pool(name="outp", bufs=1, space=bass.MemorySpace.PSUM) as outp,
            tc.tile_pool(name="epool", bufs=2) as epool,
            tc.tile_pool(name="natp", bufs=2) as natp,
            tc.tile_pool(name="tinyp", bufs=4) as tinyp,
            tc.tile_pool(name="tmpp", bufs=4) as tmpp,
        ):
            # ---- constants / weights (all DMA'd pre-packed in bf16) ----
            w_sb = const.tile([128, 3 * ED], bf16)
            nc.sync.dma_start(w_sb[:], w_d[:])
            wo_sb = const.tile([64, H, OUT_DIM], bf16)
            nc.sync.dma_start(wo_sb[:], wo_d[:])
            nlam_sb = const.tile([128, 1], f32)
            nc.sync.dma_start(nlam_sb[:], nlam_d[:])
            xsT = const.tile([128, L], bf16)
            nc.sync.dma_start(xsT[:], xsT_d[:])
            xqT = const.tile([128, QSL], bf16)
            nc.sync.dma_start(xqT[:], xqT_d[:])
            mkf = const.tile([128, NKC, QSL], bf16)
            nc.sync.dma_start(mkf[:], mT_d.rearrange("(kc p) q -> p kc q",
                                                     p=128))

            magic = const.tile([128, NQS], u32)
            nc.vector.memset(magic[:], 0x5F3759DF)

            # ---- projections (bf16 in/out, fp32 psum) ----
            qT = const.tile([128, NCH, QSL], bf16)
            kT = const.tile([128, NCH, L], bf16)
            vp = const.tile([128, NKC, H, AVP], bf16)
            for c2 in range(2):
                ps = psA.tile([128, 2, QSL], f32, tag="sps", name="ps")
                for s in range(2):
                    c = c2 * 2 + s
                    nc.tensor.matmul(ps[:, s, :],
                                     w_sb[:, c * 128:(c + 1) * 128],
                                     xqT[:], start=True, stop=True)
                nc.scalar.copy(qT[:, c2 * 2:(c2 + 1) * 2, :], ps[:])
            for c in range(NCH):
                for h2 in range(2):
                    ps = psA.tile([128, 2, QSL], f32, tag="sps", name="ps")
                    for s in range(2):
                        nc.tensor.matmul(
                            ps[:, s, :],
                            w_sb[:, ED + c * 128:ED + (c + 1) * 128],
                            xsT[:, (h2 * 2 + s) * 512:(h2 * 2 + s + 1) * 512],
                            start=True, stop=True)
                    if h2:
                        nc.scalar.copy(
                            kT[:, c, h2 * 1024:(h2 + 1) * 1024],
                            ps[:].rearrange("p s q -> p (s q)"))
                    else:
                        nc.vector.tensor_copy(
                            kT[:, c, h2 * 1024:(h2 + 1) * 1024],
                            ps[:].rearrange("p s q -> p (s q)"))
            for st2 in range(NKC // 2):
                ps = psA.tile([128, 2, QSL], f32, tag="sps", name="ps")
                for s in range(2):
                    st = st2 * 2 + s
                    nc.tensor.matmul(ps[:, s, :],
                                     xsT[:, st * 128:(st + 1) * 128],
                                     w_sb[:, 2 * ED:3 * ED],
                                     start=True, stop=True)
                for s in range(2):
                    st = st2 * 2 + s
                    if s:
                        nc.scalar.copy(
                            vp[:, st, :, 0:64],
                            ps[:, s, :].rearrange("p (h d) -> p h d", h=H))
                    else:
                        nc.vector.tensor_copy(
                            vp[:, st, :, 0:64],
                            ps[:, s, :].rearrange("p (h d) -> p h d", h=H))
            nc.vector.memset(vp[:, :, :, 64:AVP], 1.0)

            out_ps = outp.tile([128, NQS, 128], f32, tag="op", name="out_ps")
            n_outmm = [0]

            def pass1(i, kc, eb):
                c, p = divmod(i, 2)
                sps = psA.tile([128, 2, QSL], f32, tag="sps", name="sps")
                for j in range(2):
                    r = 64 * p + 32 * j
                    nc.tensor.matmul(
                        sps[:, j, :],
                        kT[r:r + 32, c, kc * 128:(kc + 1) * 128],
                        qT[r:r + 32, c, :],
                        start=True, stop=True, tile_position=(r, 0))
                nc.scalar.activation(eb[:, kc, :, :], sps[:], AF.Exp)
                mb = mkf[:, kc, :].unsqueeze(1).broadcast_to([128, 2, QSL])
                nc.vector.tensor_tensor(eb[:, kc, :, :], eb[:, kc, :, :],
                                        mb, ALU.mult)

            def av_step(i, kc, eb, av):
                g = i
                for j in range(2):
                    nc.tensor.matmul(av[j][:], vp[:, kc, g, :],
                                     eb[:, kc, j, :],
                                     start=(kc == 0), stop=(kc == NKC - 1))

            def epilogue(i, av):
                g = i
                nat = []
                for j in range(2):
                    a_sb = tmpp.tile([AVP, QSL], bf16, tag="a_sb")
                    nc.vector.tensor_copy(a_sb[:], av[j][:])
                    nt = natp.tile([128, NQS, AVP], bf16, tag=f"nat{j}",
                                   name=f"nat{j}")
                    for q in range(NQS):
                        nc.sync.dma_start_transpose(
                            nt[:, q, :], a_sb[:, q * 128:(q + 1) * 128])
                    nat.append(nt)

                r0v = tinyp.tile([128, NQS, 1], f32, tag="r0v")
                nc.vector.reciprocal(r0v[:], nat[0][:, :, 64:65])
                r1v = tinyp.tile([128, NQS, 1], f32, tag="r1v")
                nc.vector.reciprocal(r1v[:], nat[1][:, :, 64:65])
                r1p = tinyp.tile([128, NQS, 1], f32, tag="r1p")
                nc.vector.tensor_scalar(r1p[:], r1v[:], nlam_sb[:], None,
                                        ALU.mult)

                t0 = tmpp.tile([128, NQS, 64], f32, tag="t0")
                nc.vector.tensor_tensor(
                    t0[:], nat[0][:, :, 0:64],
                    r0v[:].broadcast_to([128, NQS, 64]), ALU.mult)
                t1 = tmpp.tile([128, NQS, 64], f32, tag="t1")
                nc.vector.tensor_tensor(
                    t1[:], nat[1][:, :, 0:64],
                    r1p[:].broadcast_to([128, NQS, 64]), ALU.mult)
                # at_s cols 64:128 are junk; the transposed junk rows are
                # never read (lhsT slice [0:64])
                at_s = tmpp.tile([128, NQS, 128], bf16, tag="at_s")
                nc.vector.tensor_tensor(at_s[:, :, 0:64], t0[:], t1[:],
                                        ALU.add)
                sq4 = tmpp.tile([128, NQS, 64], f32, tag="sq4")
                nc.vector.tensor_tensor(sq4[:], at_s[:, :, 0:64],
                                        at_s[:, :, 0:64], ALU.mult)
                ss4 = tinyp.tile([128, NQS], f32, tag="ss4")
                nc.vector.tensor_reduce(ss4[:], sq4[:],
                                        mybir.AxisListType.X, ALU.add)

                # rr4 = 1/sqrt(ss4/64): fast inverse sqrt on DVE
                msx = tinyp.tile([128, NQS], f32, tag="msx")
                nc.vector.tensor_scalar(msx[:], ss4[:], 1.0 / 64, None,
                                        ALU.mult)
                sh = tinyp.tile([128, NQS], u32, tag="sh")
                nc.vector.tensor_scalar(sh[:], msx[:].bitcast(u32), 1,
                                        None, ALU.logical_shift_right)
                rr4 = tinyp.tile([128, NQS], f32, tag="rr4")
                nc.vector.tensor_tensor(rr4[:].bitcast(u32), magic[:],
                                        sh[:], ALU.subtract)
                nwu = tinyp.tile([128, NQS], f32, tag="nwu")
                nww = tinyp.tile([128, NQS], f32, tag="nww")
                for _ in range(2):
                    nc.vector.tensor_tensor(nwu[:], rr4[:], rr4[:], ALU.mult)
                    nc.vector.scalar_tensor_tensor(
                        nwu[:], nwu[:], 0.5, msx[:], ALU.mult, ALU.mult)
                    nc.vector.tensor_scalar(nww[:], nwu[:], -1.0, 1.5,
                                            ALU.mult, ALU.add)
                    nc.vector.tensor_tensor(rr4[:], rr4[:], nww[:], ALU.mult)

                nc.vector.tensor_tensor(
                    at_s[:, :, 0:64], at_s[:, :, 0:64],
                    rr4[:].unsqueeze(2).broadcast_to([128, NQS, 64]),
                    ALU.mult)
                for q in range(NQS):
                    atT = tmpp.tile([128, 128], bf16, tag="atT")
                    nc.sync.dma_start_transpose(atT[:], at_s[:, q, :])
                    # single accumulation group for the whole bank: start
                    # clears has_written bank-wide
                    nc.tensor.matmul(out_ps[:, q, :], atT[0:64, :],
                                     wo_sb[:, g, :],
                                     start=(n_outmm[0] == 0),
                                     stop=(n_outmm[0] == H * NQS - 1))
                    n_outmm[0] += 1

            # ---- attention: software-pipelined over 8 pairs ----
            ebufs, avbufs = {}, {}
            for i in range(H + 1):
                if i < H:
                    eb = epool.tile([128, NKC, 2, QSL], bf16, tag="eall",
                                    name=f"eall{i}")
                    ebufs[i] = eb
                    avbufs[i] = [avp.tile([AVP, QSL], f32, tag=f"av{j}",
                                          name=f"av{j}_{i}")
                                 for j in range(2)]
                for kc in range(NKC):
                    if i < H:
                        pass1(i, kc, ebufs[i])
                    if i >= 1:
                        av_step(i - 1, kc, ebufs[i - 1], avbufs[i - 1])
                if i >= 1:
                    epilogue(i - 1, avbufs[i - 1])
                    del ebufs[i - 1], avbufs[i - 1]

            out_sb = const.tile([128, NQS, 128], f32)
            nc.vector.tensor_copy(out_sb[:], out_ps[:])
            nc.sync.dma_start(out_d.rearrange("(s p) o -> p s o", p=128),
                              out_sb[:])

    nc.compile()
    return nc


def kernel(**inputs):
    import ml_dtypes
    from concourse.bass_utils import run_bass_kernel_spmd

    bfdt = ml_dtypes.bfloat16

    x = np.asarray(inputs["x"], np.float32)
    mask = np.asarray(inputs["mask_2d"])
    Wq = np.asarray(inputs["Wq"], np.float32)
    Wkv = np.asarray(inputs["Wkv"], np.float32)
    Wout = np.asarray(inputs["Wout"], np.float32)
    lq1 = np.asarray(inputs["lambda_q1"], np.float32)
    lk1 = np.asarray(inputs["lambda_k1"], np.float32)
    lq2 = np.asarray(inputs["lambda_q2"], np.float32)
    lk2 = np.asarray(inputs["lambda_k2"], np.float32)
    gamma = np.asarray(inputs["gamma"], np.float32)

    lam = float(np.exp(np.sum(lq1 * lk1)) - np.exp(np.sum(lq2 * lk2))
                + LAMBDA_INIT)
    Wq_s = (Wq * DH ** -0.5).astype(np.float32)
    Wk = Wkv[:, :ED]
    Wv = Wkv[:, ED:]

    def pack_heads4(Wm):
        # chunk c (128 cols) holds heads 4c..4c+3 at col offsets 0/32/64/96
        out = np.empty((IN_DIM, NCH * 128), np.float32)
        for n in range(NH):
            c, r = divmod(n, 4)
            out[:, c * 128 + r * 32:c * 128 + r * 32 + 32] = \
                Wm[:, n * DH:(n + 1) * DH]
        return out

    W = np.concatenate([pack_heads4(Wq_s), pack_heads4(Wk), Wv],
                       axis=1).astype(bfdt)
    gs = (gamma * (1.0 - LAMBDA_INIT)).astype(np.float32)
    Wog = (Wout * np.tile(gs, H)[:, None])
    wo = np.ascontiguousarray(
        Wog.reshape(H, 64, OUT_DIM).transpose(1, 0, 2)).astype(bfdt)
    nlam = np.full((128, 1), -lam, np.float32)

    xsT = [np.ascontiguousarray(x[b, 0].T).astype(bfdt) for b in range(B)]
    maskT = [np.ascontiguousarray(mask[b].T.astype(np.float32)).astype(bfdt)
             for b in range(B)]

    if "nc" not in _CACHE:
        _CACHE["nc"] = _build_program()
    nc = _CACHE["nc"]

    in_maps = []
    for core in range(8):
        b, qc = divmod(core, 4)
        in_maps.append({
            "xsT": xsT[b],
            "xqT": np.ascontiguousarray(
                xsT[b][:, qc * QSL:(qc + 1) * QSL]),
            "maskT": np.ascontiguousarray(
                maskT[b][:, qc * QSL:(qc + 1) * QSL]),
            "w": W,
            "wo": wo,
            "nlam": nlam,
        })

    r = run_bass_kernel_spmd(nc, in_maps, list(range(8)))
    _CACHE["last_results"] = r
    res = r.results

    out = np.empty((B, 1, L, OUT_DIM), np.float32)
    for core in range(8):
        b, qc = divmod(core, 4)
        out[b, 0, qc * QSL:(qc + 1) * QSL, :] = res[core]["out"]
    return out
